# revision 15
# baseline (speedup 1.0000x reference)
"""Trainium2 Bass kernel for the additive-attention problem.

reference math:
    rec[b,h]    = sum_r rnn_state[b,r] * W_rec[h,r]
    scores[t,b] = sum_h tanh(enc[t,b,h] + rec[b,h]) * w_score[h] + b_score + mask[t,b]
    out         = softmax(scores, axis=t)          # (T, B) float32

Sharding: data-parallel over B across 8 cores (4 batch columns per core).
Everything is core-local (softmax is over T), so no collectives.

Per-core pipeline v2 (T=4096, BL=4, H=512) - the big tensor never touches
the PE array (f32 LDWEIGHTS is 4-pass and made a transpose-based design
TensorE-bound at ~267us):
  - DMA enc tile (256 t rows) -> SBUF natural layout (p=t%128, f=(tsub,b,h))
  - rec pre-add split across VectorE (tsub=0) and GpSimd (tsub=1)
  - ScalarE tanh f32 -> bf16
  - VectorE fused multiply+reduce (tensor_tensor_reduce, bf16 2x mode)
    against broadcast w_score -> scores_all (128, (i,tsub,b)) f32
  - mask added with one 128x128 tensor_tensor
  - ScalarE exp; one PE transpose -> (p=(i,tsub,b), f=t%128); VectorE row
    sums; block-mask matmul broadcasts per-b totals; reciprocal;
    tensor_scalar_mul; DMA out as (BL, T).
b_score cancels in softmax and is ignored.  No max-subtraction needed:
|scores| <= ||w_score||_1 + o(1) <~ 25, safely inside f32 exp range.
bf16 is used only after tanh (values in [-1,1]); observed rel err ~1e-3.
"""

import numpy as np

T, B, H, R = 4096, 32, 512, 512
NCORES = 8
BL = B // NCORES          # 4 local batch columns
TT = 256                  # t rows per tile
NTILES = T // TT          # 16
TSUB = TT // 128          # 2
HC = H // 128             # 4 h-chunks (rec matmul only)

_GRAPH = None


def _build_graph():
    import concourse.bass as bass
    import concourse.tile as tile
    from concourse import bacc, mybir
    from concourse.masks import make_identity

    f32 = mybir.dt.float32
    bf16 = mybir.dt.bfloat16
    nc = bacc.Bacc()

    enc = nc.declare_dram_parameter("enc", [T, BL, H], f32, isOutput=False)
    maskd = nc.declare_dram_parameter("maskd", [T, BL], f32, isOutput=False)
    rnnT = nc.declare_dram_parameter("rnnT", [R, BL], f32, isOutput=False)
    wrecT = nc.declare_dram_parameter("wrecT", [R, H], f32, isOutput=False)
    wscb = nc.declare_dram_parameter("wscb", [128, H], f32, isOutput=False)
    m4d = nc.declare_dram_parameter("m4", [128, 128], f32, isOutput=False)
    out = nc.declare_dram_parameter("out", [BL, T], f32, isOutput=True)


    with tile.TileContext(nc) as tc:
        with (
            tc.tile_pool(name="singles", bufs=1) as singles,
            tc.tile_pool(name="xpool", bufs=4) as xpool,
            tc.tile_pool(name="ypool", bufs=2) as ypool,
            tc.tile_pool(name="scratch", bufs=2) as scratch,
            tc.tile_pool(name="spool", bufs=2, space="PSUM") as spool,
        ):
            # ---------- constants / setup ----------
            ident = singles.tile([128, 128], f32)
            make_identity(nc, ident[:])

            m4 = singles.tile([128, 128], f32)
            nc.sync.dma_start(out=m4[:], in_=m4d[:])

            # w_score broadcast to all partitions, converted to bf16
            w_f32 = singles.tile([128, H], f32)
            nc.scalar.dma_start(out=w_f32[:], in_=wscb[:])
            w_bf = singles.tile([128, H], bf16)
            nc.vector.tensor_copy(out=w_bf[:], in_=w_f32[:])
            w8_bf = singles.tile([128, TSUB, BL, H], bf16)
            for ts in range(TSUB):
                for b in range(BL):
                    nc.vector.tensor_copy(out=w8_bf[:, ts, b, :], in_=w_f32[:])

            # mask in natural layout: (p=t%128, f=(i*tsub, b))
            mask_sb = singles.tile([128, NTILES * TSUB, BL], f32)
            nc.sync.dma_start(
                out=mask_sb[:], in_=maskd.rearrange("(its p) b -> p its b", p=128)
            )

            # rec = rnn @ W_rec.T   via 4 accumulating matmuls over r-chunks
            rnn_sb = singles.tile([128, HC, BL], f32)
            nc.scalar.dma_start(
                out=rnn_sb[:], in_=rnnT.rearrange("(rc p) b -> p rc b", p=128)
            )
            wrec_sb = singles.tile([128, HC, H], f32)
            nc.sync.dma_start(
                out=wrec_sb[:], in_=wrecT.rearrange("(rc p) h -> p rc h", p=128)
            )
            rec_ps = spool.tile([BL, H], f32, tag="scores")
            for rc in range(HC):
                nc.tensor.matmul(
                    rec_ps[:],
                    lhsT=rnn_sb[:, rc, :],
                    rhs=wrec_sb[:, rc, :],
                    start=(rc == 0),
                    stop=(rc == HC - 1),
                )
            rec_sb4 = singles.tile([BL, H], f32)
            nc.vector.tensor_copy(out=rec_sb4[:], in_=rec_ps[:])
            # broadcast (BL,H) -> (128, BL, H): one-hot row-selector matmuls
            sel = singles.tile([BL, BL, 128], f32)
            nc.gpsimd.memset(sel[:], 0.0)
            nc.gpsimd.affine_select(
                out=sel[:],
                in_=sel[:],
                compare_op=mybir.AluOpType.not_equal,
                fill=1.0,
                base=0,
                # sel[k, b, m] = (k - b) != 0 ? 0.0 : 1.0
                pattern=[[-1, BL], [0, 128]],
                channel_multiplier=1,
            )
            rec_rep = singles.tile([128, BL, H], f32)
            for b in range(BL):
                rb_ps = spool.tile([128, H], f32, tag="scores")
                nc.tensor.matmul(
                    rb_ps[:],
                    lhsT=sel[:, b, :],
                    rhs=rec_sb4[:],
                    start=True,
                    stop=True,
                )
                nc.vector.tensor_copy(out=rec_rep[:, b, :], in_=rb_ps[:])

            scores_all = singles.tile([128, NTILES * TSUB * BL], f32)  # (128,128)

            # ---------- main loop over t tiles ----------
            import os as _os
            preadd = _os.environ.get("K_PREADD", "dma")
            encv = enc.rearrange("(i ts p) b h -> i p ts (b h)", p=128, ts=TSUB)
            vunit = 0
            for i in range(NTILES):
                X = xpool.tile([128, TSUB, BL, H], f32)
                if preadd == "dma":
                    # prefill with rec, then gpsimd accum-DMA adds enc on top
                    for ts in range(TSUB):
                        nc.gpsimd.tensor_copy(out=X[:, ts], in_=rec_rep[:])
                    nc.gpsimd.dma_start(
                        out=X[:], in_=encv[i], accum_op=mybir.AluOpType.add
                    )
                else:
                    if _os.environ.get("K_SPLITQ", "1") == "1":
                        ev = encv[i].rearrange("p ts c -> p ts c")
                        nc.sync.dma_start(out=X[:, 0], in_=ev[:, 0])
                        nc.scalar.dma_start(out=X[:, 1], in_=ev[:, 1])
                    else:
                        nc.sync.dma_start(out=X[:], in_=encv[i])
                    for ts in range(TSUB):
                        if preadd == "gv" and vunit % 3 != 0:
                            nc.gpsimd.tensor_add(
                                out=X[:, ts], in0=X[:, ts], in1=rec_rep[:]
                            )
                        else:
                            nc.vector.tensor_add(
                                out=X[:, ts], in0=X[:, ts], in1=rec_rep[:]
                            )
                        vunit += 1

                Y = ypool.tile([128, TSUB, BL, H], bf16)
                nc.scalar.activation(
                    out=Y[:],
                    in_=X[:],
                    func=mybir.ActivationFunctionType.Tanh,
                )

                # prod = Y * w  (bf16 2x); reduction split between S and V:
                # first K_SRED of the 8 (ts,b) units via ScalarE activation
                # accum, the rest via V 2-level add tree + reduce.
                nsred = int(_os.environ.get("K_SRED", "4"))
                prod = scratch.tile([128, TSUB, BL, H], bf16, tag="prod")
                nc.vector.tensor_mul(out=prod[:], in0=Y[:], in1=w8_bf[:])
                base = i * TSUB * BL
                units = [(ts, b) for ts in range(TSUB) for b in range(BL)]
                for u, (ts, b) in enumerate(units[:nsred]):
                    dummy = scratch.tile([128, H], bf16, tag="sdump")
                    nc.scalar.activation(
                        out=dummy[:],
                        in_=prod[:, ts, b, :],
                        func=mybir.ActivationFunctionType.Copy,
                        accum_out=scores_all[:, base + u : base + u + 1],
                    )
                if nsred < TSUB * BL:
                    # V path over the remaining units (contiguous tail)
                    rest = prod[:].rearrange("p ts b h -> p (ts b) h")[
                        :, nsred:, :
                    ]
                    h2, h4 = H // 2, H // 4
                    nc.vector.tensor_add(
                        out=rest[:, :, :h2],
                        in0=rest[:, :, :h2],
                        in1=rest[:, :, h2:],
                    )
                    nc.vector.tensor_add(
                        out=rest[:, :, :h4],
                        in0=rest[:, :, :h4],
                        in1=rest[:, :, h4:h2],
                    )
                    nc.vector.tensor_reduce(
                        out=scores_all[:, base + nsred : base + TSUB * BL],
                        in_=rest[:, :, :h4],
                        axis=mybir.AxisListType.X,
                        op=mybir.AluOpType.add,
                    )

            # ---------- mask, exp, softmax normalization, output ----------
            nc.vector.tensor_add(
                out=scores_all[:],
                in0=scores_all[:],
                in1=mask_sb[:].rearrange("p a b -> p (a b)"),
            )
            E = singles.tile([128, 128], f32)
            nc.scalar.activation(
                out=E[:], in_=scores_all[:],
                func=mybir.ActivationFunctionType.Exp,
            )
            # transpose: (p=t%128, f=(i,ts,b)) -> (p=(i,ts,b), f=t%128)
            attT = spool.tile([128, 128], f32, tag="scores")
            nc.tensor.transpose(out=attT[:], in_=E[:], identity=ident[:])
            row_sums = singles.tile([128, 1], f32)
            nc.vector.tensor_reduce(
                out=row_sums[:], in_=attT[:], axis=mybir.AxisListType.X,
                op=mybir.AluOpType.add,
            )
            denom = spool.tile([128, 1], f32, tag="scores")
            nc.tensor.matmul(
                denom[:], lhsT=m4[:], rhs=row_sums[:], start=True, stop=True
            )
            recip = singles.tile([128, 1], f32)
            nc.vector.reciprocal(out=recip[:], in_=denom[:])
            att_out = singles.tile([128, 128], f32)
            nc.vector.tensor_scalar_mul(
                out=att_out[:], in0=attT[:], scalar1=recip[:]
            )
            # partition p = (i, ts, b) holds 128 contiguous t values for col b
            nc.sync.dma_start(
                out=out.rearrange("b (its tp) -> its b tp", tp=128),
                in_=att_out[:],
            )

    nc.compile()
    return nc


def _get_graph():
    global _GRAPH
    if _GRAPH is None:
        _GRAPH = _build_graph()
    return _GRAPH


def make_in_maps(enc, mask, rnn_state, W_rec, w_score):
    enc = np.ascontiguousarray(enc, dtype=np.float32)
    wrecT = np.ascontiguousarray(W_rec.T, dtype=np.float32)
    wscb = np.ascontiguousarray(
        np.broadcast_to(w_score.astype(np.float32)[None, :], (128, H))
    )
    m4 = (np.arange(128)[:, None] % BL == np.arange(128)[None, :] % BL).astype(
        np.float32
    )
    in_maps = []
    for c in range(NCORES):
        sl = slice(c * BL, (c + 1) * BL)
        in_maps.append(
            {
                "enc": np.ascontiguousarray(enc[:, sl, :]),
                "maskd": np.ascontiguousarray(mask[:, sl].astype(np.float32)),
                "rnnT": np.ascontiguousarray(rnn_state[sl].T.astype(np.float32)),
                "wrecT": wrecT,
                "wscb": wscb,
                "m4": m4,
            }
        )
    return in_maps


def kernel(
    encoded_contribution,
    mask,
    rnn_state,
    prev_att_weights,
    W_rec,
    w_score,
    b_score,
):
    from concourse.bass_utils import run_bass_kernel_spmd

    nc = _get_graph()
    in_maps = make_in_maps(
        np.asarray(encoded_contribution),
        np.asarray(mask),
        np.asarray(rnn_state),
        np.asarray(W_rec),
        np.asarray(w_score),
    )
    res = run_bass_kernel_spmd(nc, in_maps, list(range(NCORES)))
    outs = [np.asarray(res.results[c]["out"]) for c in range(NCORES)]
    return np.concatenate([o.T for o in outs], axis=1).astype(np.float32)


# revision 16
# speedup vs baseline: 1.9910x; 1.9910x over previous
"""Trainium2 Bass kernel for the additive-attention problem.

reference math:
    rec[b,h]    = sum_r rnn_state[b,r] * W_rec[h,r]
    scores[t,b] = sum_h tanh(enc[t,b,h] + rec[b,h]) * w_score[h] + b_score + mask[t,b]
    out         = softmax(scores, axis=t)          # (T, B) float32

Sharding: data-parallel over B across 8 cores (4 batch columns per core).
Everything is core-local (softmax is over T), so no collectives.

Per-core pipeline v2 (T=4096, BL=4, H=512) - the big tensor never touches
the PE array (f32 LDWEIGHTS is 4-pass and made a transpose-based design
TensorE-bound at ~267us):
  - DMA enc tile (256 t rows) -> SBUF natural layout (p=t%128, f=(tsub,b,h))
  - rec pre-add split across VectorE (tsub=0) and GpSimd (tsub=1)
  - ScalarE tanh f32 -> bf16
  - VectorE fused multiply+reduce (tensor_tensor_reduce, bf16 2x mode)
    against broadcast w_score -> scores_all (128, (i,tsub,b)) f32
  - mask added with one 128x128 tensor_tensor
  - ScalarE exp; one PE transpose -> (p=(i,tsub,b), f=t%128); VectorE row
    sums; block-mask matmul broadcasts per-b totals; reciprocal;
    tensor_scalar_mul; DMA out as (BL, T).
b_score cancels in softmax and is ignored.  No max-subtraction needed:
|scores| <= ||w_score||_1 + o(1) <~ 25, safely inside f32 exp range.
bf16 is used only after tanh (values in [-1,1]); observed rel err ~1e-3.
"""

import numpy as np

T, B, H, R = 4096, 32, 512, 512
NCORES = 8
BL = B // NCORES          # 4 local batch columns
TT = 256                  # t rows per tile
NTILES = T // TT          # 16
TSUB = TT // 128          # 2
HC = H // 128             # 4 h-chunks (rec matmul only)

_GRAPH = None


def _build_graph():
    import concourse.bass as bass
    import concourse.tile as tile
    from concourse import bacc, mybir
    from concourse.masks import make_identity

    f32 = mybir.dt.float32
    bf16 = mybir.dt.bfloat16
    nc = bacc.Bacc()

    enc = nc.declare_dram_parameter("enc", [T, BL, H], f32, isOutput=False)
    maskd = nc.declare_dram_parameter("maskd", [T, BL], f32, isOutput=False)
    rnnT = nc.declare_dram_parameter("rnnT", [R, BL], f32, isOutput=False)
    wrecT = nc.declare_dram_parameter("wrecT", [R, H], f32, isOutput=False)
    wscb = nc.declare_dram_parameter("wscb", [128, H], f32, isOutput=False)
    m4d = nc.declare_dram_parameter("m4", [128, 128], f32, isOutput=False)
    out = nc.declare_dram_parameter("out", [BL, T], f32, isOutput=True)


    with tile.TileContext(nc) as tc:
        with (
            tc.tile_pool(name="singles", bufs=1) as singles,
            tc.tile_pool(name="xpool", bufs=4) as xpool,
            tc.tile_pool(name="ypool", bufs=2) as ypool,
            tc.tile_pool(name="scratch", bufs=2) as scratch,
            tc.tile_pool(name="spool", bufs=2, space="PSUM") as spool,
        ):
            # ---------- constants / setup ----------
            ident = singles.tile([128, 128], f32)
            make_identity(nc, ident[:])

            m4 = singles.tile([128, 128], f32)
            nc.sync.dma_start(out=m4[:], in_=m4d[:])

            # w_score broadcast to all partitions, converted to bf16
            w_f32 = singles.tile([128, H], f32)
            nc.scalar.dma_start(out=w_f32[:], in_=wscb[:])
            w_bf = singles.tile([128, H], bf16)
            nc.vector.tensor_copy(out=w_bf[:], in_=w_f32[:])
            w8_bf = singles.tile([128, TSUB, BL, H], bf16)
            for ts in range(TSUB):
                for b in range(BL):
                    nc.vector.tensor_copy(out=w8_bf[:, ts, b, :], in_=w_f32[:])

            # mask in natural layout: (p=t%128, f=(i*tsub, b))
            mask_sb = singles.tile([128, NTILES * TSUB, BL], f32)
            nc.sync.dma_start(
                out=mask_sb[:], in_=maskd.rearrange("(its p) b -> p its b", p=128)
            )

            # rec = rnn @ W_rec.T   via 4 accumulating matmuls over r-chunks
            rnn_sb = singles.tile([128, HC, BL], f32)
            nc.scalar.dma_start(
                out=rnn_sb[:], in_=rnnT.rearrange("(rc p) b -> p rc b", p=128)
            )
            wrec_sb = singles.tile([128, HC, H], f32)
            nc.sync.dma_start(
                out=wrec_sb[:], in_=wrecT.rearrange("(rc p) h -> p rc h", p=128)
            )
            rec_ps = spool.tile([BL, H], f32, tag="scores")
            for rc in range(HC):
                nc.tensor.matmul(
                    rec_ps[:],
                    lhsT=rnn_sb[:, rc, :],
                    rhs=wrec_sb[:, rc, :],
                    start=(rc == 0),
                    stop=(rc == HC - 1),
                )
            rec_sb4 = singles.tile([BL, H], f32)
            nc.vector.tensor_copy(out=rec_sb4[:], in_=rec_ps[:])
            # broadcast (BL,H) -> (128, BL, H): one-hot row-selector matmuls
            sel = singles.tile([BL, BL, 128], f32)
            nc.gpsimd.memset(sel[:], 0.0)
            nc.gpsimd.affine_select(
                out=sel[:],
                in_=sel[:],
                compare_op=mybir.AluOpType.not_equal,
                fill=1.0,
                base=0,
                # sel[k, b, m] = (k - b) != 0 ? 0.0 : 1.0
                pattern=[[-1, BL], [0, 128]],
                channel_multiplier=1,
            )
            rec_rep = singles.tile([128, BL, H], f32)
            for b in range(BL):
                rb_ps = spool.tile([128, H], f32, tag="scores")
                nc.tensor.matmul(
                    rb_ps[:],
                    lhsT=sel[:, b, :],
                    rhs=rec_sb4[:],
                    start=True,
                    stop=True,
                )
                nc.vector.tensor_copy(out=rec_rep[:, b, :], in_=rb_ps[:])

            scores_all = singles.tile([128, NTILES * TSUB * BL], f32)  # (128,128)

            # ---------- main loop over t tiles ----------
            import os as _os
            preadd = _os.environ.get("K_PREADD", "v")
            encv = enc.rearrange("(i ts p) b h -> i p ts (b h)", p=128, ts=TSUB)
            vunit = 0
            for i in range(NTILES):
                X = xpool.tile([128, TSUB, BL, H], f32)
                if preadd == "dma":
                    # prefill with rec, then gpsimd accum-DMA adds enc on top
                    for ts in range(TSUB):
                        nc.gpsimd.tensor_copy(out=X[:, ts], in_=rec_rep[:])
                    nc.gpsimd.dma_start(
                        out=X[:], in_=encv[i], accum_op=mybir.AluOpType.add
                    )
                else:
                    if _os.environ.get("K_SPLITQ", "1") == "1":
                        ev = encv[i].rearrange("p ts c -> p ts c")
                        nc.sync.dma_start(out=X[:, 0], in_=ev[:, 0])
                        nc.scalar.dma_start(out=X[:, 1], in_=ev[:, 1])
                    else:
                        nc.sync.dma_start(out=X[:], in_=encv[i])
                    for ts in range(TSUB):
                        if preadd == "gv" and vunit % 3 != 0:
                            nc.gpsimd.tensor_add(
                                out=X[:, ts], in0=X[:, ts], in1=rec_rep[:]
                            )
                        else:
                            nc.vector.tensor_add(
                                out=X[:, ts], in0=X[:, ts], in1=rec_rep[:]
                            )
                        vunit += 1

                Y = ypool.tile([128, TSUB, BL, H], bf16)
                nc.scalar.activation(
                    out=Y[:],
                    in_=X[:],
                    func=mybir.ActivationFunctionType.Tanh,
                )

                # prod = Y * w  (bf16 2x); reduction split between S and V:
                # first K_SRED of the 8 (ts,b) units via ScalarE activation
                # accum, the rest via V 2-level add tree + reduce.
                nsred = int(_os.environ.get("K_SRED", "4"))
                prod = scratch.tile([128, TSUB, BL, H], bf16, tag="prod")
                nc.vector.tensor_mul(out=prod[:], in0=Y[:], in1=w8_bf[:])
                base = i * TSUB * BL
                units = [(ts, b) for ts in range(TSUB) for b in range(BL)]
                for u, (ts, b) in enumerate(units[:nsred]):
                    dummy = scratch.tile([128, H], bf16, tag="sdump")
                    nc.scalar.activation(
                        out=dummy[:],
                        in_=prod[:, ts, b, :],
                        func=mybir.ActivationFunctionType.Copy,
                        accum_out=scores_all[:, base + u : base + u + 1],
                    )
                if nsred < TSUB * BL:
                    # V path over the remaining units (contiguous tail)
                    rest = prod[:].rearrange("p ts b h -> p (ts b) h")[
                        :, nsred:, :
                    ]
                    h2, h4 = H // 2, H // 4
                    nc.vector.tensor_add(
                        out=rest[:, :, :h2],
                        in0=rest[:, :, :h2],
                        in1=rest[:, :, h2:],
                    )
                    nc.vector.tensor_add(
                        out=rest[:, :, :h4],
                        in0=rest[:, :, :h4],
                        in1=rest[:, :, h4:h2],
                    )
                    nc.vector.tensor_reduce(
                        out=scores_all[:, base + nsred : base + TSUB * BL],
                        in_=rest[:, :, :h4],
                        axis=mybir.AxisListType.X,
                        op=mybir.AluOpType.add,
                    )

            # ---------- mask, exp, softmax normalization, output ----------
            nc.vector.tensor_add(
                out=scores_all[:],
                in0=scores_all[:],
                in1=mask_sb[:].rearrange("p a b -> p (a b)"),
            )
            E = singles.tile([128, 128], f32)
            nc.scalar.activation(
                out=E[:], in_=scores_all[:],
                func=mybir.ActivationFunctionType.Exp,
            )
            # transpose: (p=t%128, f=(i,ts,b)) -> (p=(i,ts,b), f=t%128)
            attT = spool.tile([128, 128], f32, tag="scores")
            nc.tensor.transpose(out=attT[:], in_=E[:], identity=ident[:])
            row_sums = singles.tile([128, 1], f32)
            nc.vector.tensor_reduce(
                out=row_sums[:], in_=attT[:], axis=mybir.AxisListType.X,
                op=mybir.AluOpType.add,
            )
            denom = spool.tile([128, 1], f32, tag="scores")
            nc.tensor.matmul(
                denom[:], lhsT=m4[:], rhs=row_sums[:], start=True, stop=True
            )
            recip = singles.tile([128, 1], f32)
            nc.vector.reciprocal(out=recip[:], in_=denom[:])
            att_out = singles.tile([128, 128], f32)
            nc.vector.tensor_scalar_mul(
                out=att_out[:], in0=attT[:], scalar1=recip[:]
            )
            # partition p = (i, ts, b) holds 128 contiguous t values for col b
            nc.sync.dma_start(
                out=out.rearrange("b (its tp) -> its b tp", tp=128),
                in_=att_out[:],
            )

    nc.compile()
    return nc


def _get_graph():
    global _GRAPH
    if _GRAPH is None:
        _GRAPH = _build_graph()
    return _GRAPH


def make_in_maps(enc, mask, rnn_state, W_rec, w_score):
    enc = np.ascontiguousarray(enc, dtype=np.float32)
    wrecT = np.ascontiguousarray(W_rec.T, dtype=np.float32)
    wscb = np.ascontiguousarray(
        np.broadcast_to(w_score.astype(np.float32)[None, :], (128, H))
    )
    m4 = (np.arange(128)[:, None] % BL == np.arange(128)[None, :] % BL).astype(
        np.float32
    )
    in_maps = []
    for c in range(NCORES):
        sl = slice(c * BL, (c + 1) * BL)
        in_maps.append(
            {
                "enc": np.ascontiguousarray(enc[:, sl, :]),
                "maskd": np.ascontiguousarray(mask[:, sl].astype(np.float32)),
                "rnnT": np.ascontiguousarray(rnn_state[sl].T.astype(np.float32)),
                "wrecT": wrecT,
                "wscb": wscb,
                "m4": m4,
            }
        )
    return in_maps


def kernel(
    encoded_contribution,
    mask,
    rnn_state,
    prev_att_weights,
    W_rec,
    w_score,
    b_score,
):
    from concourse.bass_utils import run_bass_kernel_spmd

    nc = _get_graph()
    in_maps = make_in_maps(
        np.asarray(encoded_contribution),
        np.asarray(mask),
        np.asarray(rnn_state),
        np.asarray(W_rec),
        np.asarray(w_score),
    )
    res = run_bass_kernel_spmd(nc, in_maps, list(range(NCORES)))
    outs = [np.asarray(res.results[c]["out"]) for c in range(NCORES)]
    return np.concatenate([o.T for o in outs], axis=1).astype(np.float32)
